# revision 7
# baseline (speedup 1.0000x reference)
"""Dilated self-attention TRN2 kernel (nn_DilatedSelfAttention).

Problem (hardcoded — self-contained):
  x (4, 8192, 128) f32; Wq/Wk/Wv (128,128) f32; indices (14336) i64.
  WS=[2048,4096,8192], RS=[1,2,4], HEAD_IDX=1 -> 7 segments of 2048 per batch:
    seg0..3: windows [2048t, 2048(t+1))           (stride 1)
    seg4:    1 + 2*i, i<2048   (odd of [0,4096))  (stride 2)
    seg5:    4097 + 2*i        (odd of [4096,8192))
    seg6:    1 + 4*i           (p%4==1)           (stride 4)
  Each segment: causal softmax attention (per-segment row max subtracted),
  outputs mixed position-wise weighted by softmax denominators:
    out[p] = sum_seg (expS @ v)[p] / sum_seg denom[p]   (with per-seg max shifts
    folded into both numerator and denominator — matches reference exactly).

Sharding: core pair (2b, 2b+1) owns batch b. Each segment is split into two
half-pieces by query 128-tile parity (delta=0: even qtiles, delta=1: odd).
Every core runs SEVEN structurally identical pieces (uniform SPMD program);
the only per-core data differences are the gathered inputs, the diag masks,
and a dynamic column offset (128*delta) for the output scatter.

Per piece (segment context S=2048, local queries QL=1024 in 8 slots of 128):
  q' = x_seg @ (Wq Wk^T/sqrt(C))  [f32r]     k == x_seg itself
  slot j: S-row = q'_j @ x^T over 256*(j+1) keys [f32r matmuls into PSUM],
    additive -1e9 diag/pad mask via identity@mask matmul,
    rowmax (DVE) -> exp with bias=-mx, fused denom (ACT accum_out) -> E f16,
    blocked DMA-xbar transpose E -> ET[k-chunk, local q] (zero-padded region
    persists from a one-time memset),
  EV: out^T[c, q] accumulated over k-chunks (v f16 stationary, ET moving),
  scatter-add out^T columns / denoms into batch-position accumulators
  (gpsimd adds at dynamic strided offsets).
Pair ReduceScatter sums the two cores' accumulators; each core normalizes and
writes half the batch rows.
"""
import math
import os
import sys

sys.path.insert(0, "/opt/trn_rl_repo")

import numpy as np

import concourse.bass as bass
import concourse.bacc as bacc
import concourse.mybir as mybir
import concourse.tile as tile
from concourse.bass_utils import run_bass_kernel_spmd
from concourse.masks import make_identity

f32 = mybir.dt.float32
f32r = mybir.dt.float32r
f16 = mybir.dt.float16
i32 = mybir.dt.int32

B, N, C = 4, 8192, 128
S = 2048          # segment length
NCH = 16          # 128-chunks per segment
NSLOT = 8         # q-slots per piece
QL = NSLOT * 128  # 1024 local queries per piece
NPIECE = 7
NEG = -1e9

# per piece-slot-index: segment id == piece id; (base, stride) of position map
SEG_BASE = [0, 2048, 4096, 6144, 1, 4097, 1]
SEG_STRIDE = [1, 1, 1, 1, 2, 2, 4]


def build_nc():
    nc = bacc.Bacc(None, target_bir_lowering=False)

    bxT7 = nc.dram_tensor("bxT7", [NPIECE, C, S], f32, kind="ExternalInput")
    qxT7 = nc.dram_tensor("qxT7", [NPIECE, C, QL], f32, kind="ExternalInput")
    mask7 = nc.dram_tensor("mask7", [NPIECE, 128, 256], f32, kind="ExternalInput")
    beta7 = nc.dram_tensor("beta7", [1, NPIECE], i32, kind="ExternalInput")
    Mt = nc.dram_tensor("Mt", [C, C], f32, kind="ExternalInput")
    Wvt = nc.dram_tensor("Wvt", [C, C], f32, kind="ExternalInput")
    out_half = nc.dram_tensor("out_half", [N // 2, C], f32, kind="ExternalOutput")

    HALF = N // 2                      # 4096 positions per core after RS
    NUMSZ = C * HALF                   # 524288
    EXSZ = NUMSZ + HALF                # + DenT half

    with tile.TileContext(nc) as tc:
        with (
            tc.tile_pool(name="fix", bufs=1) as fix,
            tc.tile_pool(name="bx", bufs=1) as bxp,
            tc.tile_pool(name="bxr", bufs=2) as bxrp,
            tc.tile_pool(name="bx16", bufs=1) as bx16p,
            tc.tile_pool(name="qx", bufs=2) as qxp,
            tc.tile_pool(name="qpr", bufs=2) as qprp,
            tc.tile_pool(name="vsl", bufs=2) as vslp,
            tc.tile_pool(name="msk", bufs=2) as mskp,
            tc.tile_pool(name="E", bufs=2) as Ep,
            tc.tile_pool(name="small", bufs=2) as smp,
            tc.tile_pool(name="evt", bufs=1) as evtp,
            tc.tile_pool(name="spool", bufs=5, space="PSUM") as spool,
            tc.tile_pool(name="evp", bufs=2, space="PSUM") as evp,
            tc.tile_pool(name="dram", bufs=1, space="DRAM") as drp,
            tc.tile_pool(name="epi", bufs=1) as epi,
        ):
            # ---- fixed tensors ----
            ident = fix.tile([128, 128], f32)
            make_identity(nc, ident[:])
            ident_r = fix.tile([128, 128], f32r)
            nc.gpsimd.tensor_copy(ident_r[:], ident[:])

            m_f = fix.tile([C, C], f32)
            wv_f = fix.tile([C, C], f32)
            nc.sync.dma_start(m_f[:], Mt[:])
            nc.sync.dma_start(wv_f[:], Wvt[:])
            m_r = fix.tile([C, C], f32r)
            wv16 = fix.tile([C, C], f16)
            nc.gpsimd.tensor_copy(m_r[:], m_f[:])
            nc.gpsimd.tensor_copy(wv16[:], wv_f[:])

            beta_sb = fix.tile([1, NPIECE], i32)
            nc.sync.dma_start(beta_sb[:], beta7[:])

            NumT = fix.tile([C, N], f32)
            DenT = fix.tile([1, N], f32)
            ET = fix.tile([128, NCH, QL], f16)
            nc.gpsimd.memset(NumT[:], 0.0)
            nc.gpsimd.memset(DenT[:], 0.0)
            nc.vector.memset(ET[:], 0.0)

            exch_in = drp.tile([2, EXSZ], f32)
            exch_out = drp.tile([1, EXSZ], f32)

            # ---- pieces ----
            for p in range(NPIECE):
                sstr = SEG_STRIDE[p]
                sbase = SEG_BASE[p]

                bxT = bxp.tile([C, S], f32)
                qx = bxp.tile([C, QL], f32, tag="qxf")
                mskf = smp.tile([128, 256], f32, tag="mskf")
                nc.sync.dma_start(bxT[:], bxT7[p])
                nc.sync.dma_start(qx[:], qxT7[p])
                nc.sync.dma_start(mskf[:], mask7[p])

                bxr = bxrp.tile([C, S], f32r)
                bx16 = bx16p.tile([C, S], f16)
                qxr = qxp.tile([C, QL], f32r, tag="qxr")
                mask_r = mskp.tile([128, 256], f32r)
                nc.gpsimd.tensor_copy(bxr[:], bxT[:])
                nc.gpsimd.tensor_copy(bx16[:], bxT[:])
                nc.gpsimd.tensor_copy(qxr[:], qx[:])
                nc.gpsimd.tensor_copy(mask_r[:], mskf[:])

                # q' projection: q'T [c_out, QL] f32r
                qpt = qprp.tile([C, QL], f32r)
                for h in range(2):
                    qp_ps = spool.tile([128, 512], f32, tag="s")
                    nc.tensor.matmul(
                        qp_ps[:], m_r[:], qxr[:, 512 * h : 512 * h + 512],
                        start=True, stop=True,
                    )
                    nc.scalar.copy(qpt[:, 512 * h : 512 * h + 512], qp_ps[:])

                # v chunks: [k128, c128] f16, slab [128, NCH*128]
                vsl = vslp.tile([128, NCH * 128], f16)
                for cch in range(NCH):
                    v_ps = spool.tile([128, 512], f32, tag="s")
                    nc.tensor.matmul(
                        v_ps[:, 0:128],
                        bx16[:, 128 * cch : 128 * cch + 128],
                        wv16[:],
                        start=True, stop=True,
                    )
                    eng = nc.vector if cch % 2 == 0 else nc.scalar
                    if eng is nc.vector:
                        nc.vector.tensor_copy(
                            vsl[:, 128 * cch : 128 * cch + 128], v_ps[:, 0:128]
                        )
                    else:
                        nc.scalar.copy(
                            vsl[:, 128 * cch : 128 * cch + 128], v_ps[:, 0:128]
                        )

                denslab = smp.tile([128, NSLOT], f32, tag="denslab")

                # ---- softmax rows (slots) ----
                for j in range(NSLOT):
                    ext = 256 * (j + 1)
                    nt = (ext + 511) // 512  # S tiles of 512
                    stiles = []
                    for t in range(nt):
                        w = min(512, ext - 512 * t)
                        st = spool.tile([128, 512], f32, tag="s")
                        stiles.append((st, w))
                        # score matmuls, 512-wide (or 256 tail)
                        nc.tensor.matmul(
                            st[:, 0:w],
                            qpt[:, 128 * j : 128 * j + 128],
                            bxr[:, 512 * t : 512 * t + w],
                            start=True,
                            stop=not (t == nt - 1),
                        )
                    # additive diag/pad mask into last 256 columns
                    last_st, last_w = stiles[-1]
                    nc.tensor.matmul(
                        last_st[:, last_w - 256 : last_w],
                        ident_r[:],
                        mask_r[:],
                        start=False,
                        stop=True,
                    )

                    maxp = smp.tile([128, 4], f32, tag="maxp")
                    for t, (st, w) in enumerate(stiles):
                        nc.vector.tensor_reduce(
                            maxp[:, t : t + 1], st[:, 0:w],
                            axis=mybir.AxisListType.X, op=mybir.AluOpType.max,
                        )
                    negmx = smp.tile([128, 1], f32, tag="negmx")
                    nc.vector.tensor_reduce(
                        negmx[:], maxp[:, 0:nt],
                        axis=mybir.AxisListType.X, op=mybir.AluOpType.max,
                        negate=True,
                    )

                    Et = Ep.tile([128, S], f16)
                    denp = smp.tile([128, 4], f32, tag="denp")
                    for t, (st, w) in enumerate(stiles):
                        nc.scalar.activation(
                            Et[:, 512 * t : 512 * t + w],
                            st[:, 0:w],
                            mybir.ActivationFunctionType.Exp,
                            bias=negmx[:, 0:1],
                            scale=1.0,
                            accum_out=denp[:, t : t + 1],
                        )
                    nc.vector.tensor_reduce(
                        denslab[:, j : j + 1], denp[:, 0:nt],
                        axis=mybir.AxisListType.X, op=mybir.AluOpType.add,
                    )

                    # blocked transpose E[:, 0:ext] -> ET[:, 0:2(j+1), 128j:+128]
                    nc.sync.dma_start_transpose(
                        ET[:, 0 : 2 * (j + 1), 128 * j : 128 * j + 128],
                        Et[:, 0:ext],
                    )

                # ---- EV: out^T [c, q] ----
                evts = evtp.tile([C, QL], f32)
                ev_ps0 = evp.tile([128, 512], f32, tag="ev")
                ev_ps1 = evp.tile([128, 512], f32, tag="ev")
                ev_ps = [ev_ps0, ev_ps1]
                for cch in range(NCH):
                    for g in range(2):
                        if g == 0 and cch >= 8:
                            continue
                        nc.tensor.matmul(
                            ev_ps[g][:],
                            vsl[:, 128 * cch : 128 * cch + 128],
                            ET[:, cch, 512 * g : 512 * g + 512],
                            start=(cch == 0),
                            stop=(cch == (7 if g == 0 else 15)),
                        )
                for g in range(2):
                    nc.scalar.copy(evts[:, 512 * g : 512 * g + 512], ev_ps[g][:])

                # ---- denoms to free-layout: denslab [128,8] -> denrow [1, QL] ----
                dslT = evp.tile([NSLOT, 128], f32, tag="ev")
                nc.tensor.transpose(dslT[:], denslab[:, 0:NSLOT], ident[:])
                dsl_sb = smp.tile([NSLOT, 128], f32, tag="dslsb")
                nc.scalar.copy(dsl_sb[:], dslT[:])
                denrow = smp.tile([1, QL], f32, tag="denrow")
                nc.sync.dma_start(denrow[:], dsl_sb[:])

                # ---- scatter-add into NumT / DenT at dynamic offset ----
                regs = nc.alloc_registers(f"beta_{p}", engines=[mybir.EngineType.Pool])
                nc.regs_load(regs, beta_sb[0:1, p : p + 1])
                beta = nc.snap(regs, donate=True, min_val=0, max_val=128)

                numv = NumT[:, sbase :: sstr]
                denv = DenT[:, sbase :: sstr]
                for j in range(NSLOT):
                    dsl = bass.ds(beta + 256 * j, 128)
                    nc.gpsimd.tensor_tensor(
                        numv[:, dsl], numv[:, dsl],
                        evts[:, 128 * j : 128 * j + 128],
                        op=mybir.AluOpType.add,
                    )
                    nc.gpsimd.tensor_tensor(
                        denv[:, dsl], denv[:, dsl],
                        denrow[:, 128 * j : 128 * j + 128],
                        op=mybir.AluOpType.add,
                    )

            # ---- exchange: ReduceScatter over the pair ----
            for h in range(2):
                nc.sync.dma_start(
                    exch_in[h, 0:NUMSZ].rearrange("(p f) -> p f", p=C),
                    NumT[:, HALF * h : HALF * h + HALF],
                )
                nc.sync.dma_start(
                    exch_in[h, NUMSZ:EXSZ].rearrange("(p f) -> p f", p=1),
                    DenT[:, HALF * h : HALF * h + HALF],
                )
            nc.gpsimd.collective_compute(
                "ReduceScatter",
                mybir.AluOpType.add,
                replica_groups=[[0, 1], [2, 3], [4, 5], [6, 7]],
                ins=[exch_in.opt()],
                outs=[exch_out.opt()],
            )

            # ---- epilogue: normalize + transpose to [pos, c] rows ----
            nsum = epi.tile([C, HALF], f32)
            nc.sync.dma_start(
                nsum[:], exch_out[0, 0:NUMSZ].rearrange("(p f) -> p f", p=C)
            )
            d32 = epi.tile([32, 128], f32, tag="d32")
            nc.sync.dma_start(
                d32[:], exch_out[0, NUMSZ:EXSZ].rearrange("(a b) -> a b", a=32)
            )
            dT = evp.tile([128, 32], f32, tag="ev")
            nc.tensor.transpose(dT[:], d32[:], ident[0:32, 0:32])
            dT_sb = epi.tile([128, 32], f32, tag="dTsb")
            nc.scalar.copy(dT_sb[:], dT[:])
            recipD = epi.tile([128, 32], f32, tag="recipD")
            nc.vector.reciprocal(recipD[:], dT_sb[:])

            oview = out_half.rearrange("(m p) c -> p m c", p=128)
            for m in range(32):
                tp = evp.tile([128, 128], f32, tag="ev")
                ot = mskp.tile([128, 128], f32, tag="ot")
                nc.tensor.transpose(tp[:], nsum[:, 128 * m : 128 * m + 128], ident[:])
                nc.vector.tensor_scalar_mul(ot[:], tp[:], recipD[:, m : m + 1])
                nc.sync.dma_start(oview[:, m, :], ot[:])

    nc.finalize()
    return nc


# ---------------- host side ----------------

_SEG_POS = None


def _seg_positions():
    global _SEG_POS
    if _SEG_POS is None:
        segs = []
        for w, r in zip([2048, 4096, 8192], [1, 2, 4]):
            off = 1 % r
            for start in range(0, N, w):
                segs.append(np.arange(start, start + w)[off::r])
        _SEG_POS = segs  # 7 arrays of 2048
    return _SEG_POS


def _make_masks():
    q = np.arange(128)[:, None]
    k = np.arange(128)[None, :]
    tri = np.where(k <= q, 0.0, NEG).astype(np.float32)
    zero = np.zeros((128, 128), np.float32)
    full = np.full((128, 128), NEG, np.float32)
    m_even = np.concatenate([tri, full], axis=1)   # delta=0: diag chunk first
    m_odd = np.concatenate([zero, tri], axis=1)    # delta=1: diag chunk last
    return m_even, m_odd


_NC = None


def _get_nc():
    global _NC
    if _NC is None:
        _NC = build_nc()
    return _NC


def kernel(x, Wq, Wk, Wv, indices):
    x = np.asarray(x, dtype=np.float32)
    Wq = np.asarray(Wq, dtype=np.float32)
    Wk = np.asarray(Wk, dtype=np.float32)
    Wv = np.asarray(Wv, dtype=np.float32)

    M = (Wq.astype(np.float64) @ Wk.T.astype(np.float64) / math.sqrt(C)).astype(
        np.float32
    )
    m_even, m_odd = _make_masks()
    segs = _seg_positions()

    in_maps = []
    for core in range(8):
        b = core // 2
        odd_core = core % 2
        xTb = np.ascontiguousarray(x[b].T)  # (C, N)
        bxT7 = np.empty((NPIECE, C, S), np.float32)
        qxT7 = np.empty((NPIECE, C, QL), np.float32)
        mask7 = np.empty((NPIECE, 128, 256), np.float32)
        beta7 = np.empty((1, NPIECE), np.int32)
        for p in range(NPIECE):
            # delta: core even -> segs0-3 even-qtiles, segs4-6 odd; odd core flips
            delta = (0 if p < 4 else 1) ^ odd_core
            pos = segs[p]
            bxT7[p] = xTb[:, pos]
            sq = (
                128 * (2 * np.arange(NSLOT)[:, None] + delta)
                + np.arange(128)[None, :]
            ).reshape(-1)
            qxT7[p] = bxT7[p][:, sq]
            mask7[p] = m_even if delta == 0 else m_odd
            beta7[0, p] = 128 * delta
        in_maps.append(
            {
                "bxT7": bxT7,
                "qxT7": qxT7,
                "mask7": mask7,
                "beta7": beta7,
                "Mt": M,
                "Wvt": Wv,
            }
        )

    nc = _get_nc()
    res = run_bass_kernel_spmd(nc, in_maps, list(range(8))).results

    out = np.empty((B, N, C), np.float32)
    for b in range(B):
        out[b, : N // 2] = res[2 * b]["out_half"]
        out[b, N // 2 :] = res[2 * b + 1]["out_half"]
    return out


def kernel_profiled(x, Wq, Wk, Wv, indices, **trace_kwargs):
    """Like kernel() but returns (out, BassKernelResults) with trace enabled."""
    import kernel as _self
    global run_bass_kernel_spmd
    orig = run_bass_kernel_spmd
    holder = {}

    def wrapper(nc, in_maps, core_ids, **kw):
        r = orig(nc, in_maps, core_ids, trace=True, **trace_kwargs)
        holder["r"] = r
        return r

    run_bass_kernel_spmd = wrapper
    try:
        out = kernel(x, Wq, Wk, Wv, indices)
    finally:
        run_bass_kernel_spmd = orig
    return out, holder["r"]
